# revision 37
# baseline (speedup 1.0000x reference)
"""AdapterGNN distributed Trainium2 kernel (8 NeuronCores, Bass/Tile).

out = norm_dst * segsum_dst( (X*norm_src) @ Wd + norm_src*bd )[src] @ (Wg@Wu) + (bg@Wu+bu)

Src-partitioned design: each core owns the nodes (and down-projects their
features to a local fp16 h table) and the edges whose SRC it owns. Edges are
grouped by dst core into 8 streams; per stream, bulk `dma_gather` pulls
h[src_local] rows (int16 local indices) and `dma_scatter_add` accumulates them
into a per-dst-core partial table in HBM. Each stream is split into rounds so
no dst row repeats within one scatter instruction (the HW's concurrent
read-modify-write loses same-address updates). A ReduceScatter (add) then
combines the 8x8 partials so each core holds the final aggregation for its own
nodes; the tail applies norm_dst (folded into the PE transpose as a diagonal
matrix), the fused (Wg@Wu) up-projection and the bias, and stores fp16 output
that the host widens to fp32.

Self-contained: requires only numpy + concourse (+ TRN2 cores via axon).
"""

import numpy as np

import concourse.bacc as bacc
import concourse.mybir as mybir
import concourse.tile as tile
from concourse.bass_utils import run_bass_kernel_spmd

F32 = mybir.dt.float32
F16 = mybir.dt.float16
I16 = mybir.dt.int16

P = 128          # partitions
CHUNK = 1024     # edges per dma_gather: the SWDGE in-flight window (8 DMAs)
SCAP = 512       # max edges per dma_scatter_add; scatter burns 2 descs/edge
                 # must stay under the 1024-desc carveout or the ring wedges
HST = 7          # windows per h-store / agg-load batch


class Cfg:
    def __init__(self, n_nodes, n_edges, in_dim, out_dim, n_cores=8):
        self.N = n_nodes
        self.E = n_edges
        self.IN = in_dim          # 768
        self.OUT = out_dim        # 128 (must be 128)
        self.C = n_cores
        assert out_dim == P
        self.NpReal = (n_nodes + n_cores - 1) // n_cores   # real nodes per core
        self.W = (self.NpReal + P - 1) // P                # windows per core
        self.Np = self.W * P                               # padded nodes/core
        self.KC = in_dim // P                              # full K chunks (6)
        assert in_dim % P == 0
        self.KIN = self.KC + 1                             # +1 chunk for (norm,bias) row
        self.GARB = self.Np - 1                            # garbage agg slot (pad row)


def _wrap16(v):
    """Edge i -> [i%16, i//16], replicated across the 8 GPSIMD groups."""
    n = len(v)
    assert n % 16 == 0
    S = n // 16
    t = np.zeros((P, S), dtype=np.int16)
    blk = v.reshape(S, 16).T.astype(np.int16)
    for k in range(8):
        t[16 * k:16 * (k + 1), :] = blk
    return t


def host_prep(cfg, features, Wd, bd, Wg, bg, Wu, bu, src, dst):
    """Returns (in_maps, node_core, node_slot, shared)."""
    C, N, Np, W = cfg.C, cfg.N, cfg.Np, cfg.W
    src = np.asarray(src).astype(np.int64)
    dst = np.asarray(dst).astype(np.int64)
    features = np.asarray(features, dtype=np.float32)

    out_deg = np.bincount(src, minlength=N)
    in_deg = np.bincount(dst, minlength=N)
    norm_src = 1.0 / np.sqrt(np.maximum(out_deg, 1.0))
    norm_dst = 1.0 / np.sqrt(np.maximum(in_deg, 1.0))

    node_core = np.minimum(np.arange(N) // cfg.NpReal, C - 1)
    node_slot = np.arange(N, dtype=np.int64) - node_core * cfg.NpReal

    # --- per (src core, src half, dst core) streams with dup-free rounds ----
    # src half H splits the local h table so gathers of half 0 can start
    # before the second half of the down-projection finishes.
    HALF = (cfg.W // 2) * P                    # rows in h_lo
    e_sc = node_core[src]
    e_dc = node_core[dst]
    e_ss = node_slot[src]
    e_ds = node_slot[dst]
    e_hf = (e_ss >= HALF).astype(np.int64)

    # round index of each edge within its (src core, half, dst core, dst slot)
    key = ((e_sc * 2 + e_hf) * C + e_dc) * Np + e_ds
    order = np.argsort(key, kind="stable")
    ko = key[order]
    rnd_o = np.arange(len(ko)) - np.searchsorted(ko, ko)
    rnd = np.empty(len(ko), dtype=np.int64)
    rnd[order] = rnd_o

    # shared round sizes: max over src cores, 128-aligned
    NR = np.zeros((2, C), dtype=np.int64)      # rounds per (half, dst core)
    nmax = {}                                  # (h, d, r) -> max count over c
    for c in range(C):
        for h in range(2):
            for d in range(C):
                m = (e_sc == c) & (e_hf == h) & (e_dc == d)
                rcnt = np.bincount(rnd[m])
                NR[h, d] = max(NR[h, d], len(rcnt))
                for r, n in enumerate(rcnt):
                    nmax[(h, d, r)] = max(nmax.get((h, d, r), 0), int(n))
    RS_SIZES = tuple(
        tuple(
            tuple(((nmax[(h, d, r)] + 127) // 128) * 128 for r in range(NR[h, d]))
            for d in range(C)
        )
        for h in range(2)
    )
    NPAD = tuple(
        tuple(int(sum(RS_SIZES[h][d])) for d in range(C)) for h in range(2)
    )

    # --- weights ------------------------------------------------------------
    Wgu = (np.asarray(Wg, np.float64) @ np.asarray(Wu, np.float64)).astype(np.float32)
    bu2 = (np.asarray(bg, np.float64) @ np.asarray(Wu, np.float64)
           + np.asarray(bu, np.float64)).astype(np.float32)
    wd_h = np.zeros((P, cfg.KIN * cfg.OUT), dtype=np.float16)
    for cc in range(cfg.KC):
        wd_h[:, cc * cfg.OUT:(cc + 1) * cfg.OUT] = Wd[cc * P:(cc + 1) * P, :]
    wd_h[0, cfg.KC * cfg.OUT:(cfg.KC + 1) * cfg.OUT] = bd
    wgu_h = Wgu.astype(np.float16)
    bu2_h = np.zeros((P, cfg.IN), dtype=np.float16)
    bu2_h[0, :] = bu2.astype(np.float16)

    in_maps = []
    for c in range(C):
        ids = np.arange(c * cfg.NpReal, min((c + 1) * cfg.NpReal, N))
        n_real = len(ids)

        # xa: per-window contiguous [P, W*(KIN*P)] fp16
        xs = (features[ids, :] * norm_src[ids, None]).astype(np.float16)
        xa = np.zeros((P, W * cfg.KIN * P), dtype=np.float16)
        xav = xa.reshape(P, W, cfg.KIN, P)
        for w in range(W):
            lo = w * P
            hi = min(lo + P, n_real)
            nn = hi - lo
            if nn <= 0:
                break
            for cc in range(cfg.KC):
                xav[:, w, cc, :nn] = xs[lo:hi, cc * P:(cc + 1) * P].T
            xav[0, w, cfg.KC, :nn] = norm_src[ids[lo:hi]].astype(np.float16)

        # diag(norm_dst) per window for the scale-folding transpose
        diag = np.zeros((P, W * P), dtype=np.float16)
        nd = np.ones(Np, dtype=np.float32)
        nd[:n_real] = norm_dst[ids]
        for w in range(W):
            np.fill_diagonal(diag[:, w * P:(w + 1) * P], nd[w * P:(w + 1) * P])

        im = {"xa": xa, "wd": wd_h, "wgu": wgu_h, "bu2": bu2_h, "diag": diag}

        # per (half, dst core) idx streams; gather idx is local to the half
        for h in range(2):
            for d in range(C):
                m = (e_sc == c) & (e_hf == h) & (e_dc == d)
                ss, ds_, rr = e_ss[m] - h * HALF, e_ds[m], rnd[m]
                gi = np.zeros(NPAD[h][d], np.int64)          # sentinel: row 0
                di = np.full(NPAD[h][d], cfg.GARB, np.int64)  # garbage row
                off = 0
                for r, size in enumerate(RS_SIZES[h][d]):
                    sel = rr == r
                    n = int(sel.sum())
                    gi[off:off + n] = ss[sel]
                    di[off:off + n] = ds_[sel]
                    off += size
                im[f"g{h}_{d}"] = _wrap16(gi)
                im[f"s{h}_{d}"] = _wrap16(di)
        in_maps.append(im)

    return in_maps, node_core, node_slot, (NPAD, RS_SIZES)


def build_graph(cfg, shared):
    """Build the SPMD Bass graph (same for all cores)."""
    NPAD, RS_SIZES = shared
    W, Np, OUT, IN, C = cfg.W, cfg.Np, cfg.OUT, cfg.IN, cfg.C
    WLO = W // 2
    HALF = WLO * P

    nc = bacc.Bacc(None, target_bir_lowering=False)
    xa = nc.declare_dram_parameter("xa", [P, W * cfg.KIN * P], F16, False)
    wd = nc.declare_dram_parameter("wd", [P, cfg.KIN * OUT], F16, False)
    wgu = nc.declare_dram_parameter("wgu", [OUT, IN], F16, False)
    bu2 = nc.declare_dram_parameter("bu2", [P, IN], F16, False)
    diag = nc.declare_dram_parameter("diag", [P, W * P], F16, False)
    gaps = [[nc.declare_dram_parameter(f"g{h}_{d}", [P, NPAD[h][d] // 16], I16, False)
             for d in range(C)] for h in range(2)]
    saps = [[nc.declare_dram_parameter(f"s{h}_{d}", [P, NPAD[h][d] // 16], I16, False)
             for d in range(C)] for h in range(2)]
    out = nc.declare_dram_parameter("out", [Np, IN], F16, True)

    with tile.TileContext(nc) as tc:
        with (
            tc.tile_pool(name="dram", bufs=1, space="DRAM") as dram,
            tc.tile_pool(name="const", bufs=1) as const,
            tc.tile_pool(name="xat", bufs=4) as xap,
            tc.tile_pool(name="hsb", bufs=2) as hsb,
            tc.tile_pool(name="dpsum", bufs=2, space="PSUM") as dpsum,
            tc.tile_pool(name="idxp", bufs=1) as idxp,
            tc.tile_pool(name="gsb", bufs=3) as gsb,
            tc.tile_pool(name="asb", bufs=2) as asb,
            tc.tile_pool(name="tpsum", bufs=2, space="PSUM") as tpsum,
            tc.tile_pool(name="atb", bufs=3) as atb,
            tc.tile_pool(name="opsum", bufs=2, space="PSUM") as opsum,
            tc.tile_pool(name="osb", bufs=3) as osb,
        ):
            h_lo = dram.tile([HALF, OUT], F16)
            h_hi = dram.tile([Np - HALF, OUT], F16)
            h_half = [h_lo, h_hi]
            aggs = dram.tile([C * Np, OUT], F16)
            rs_out = dram.tile([Np, OUT], F16)

            # idx tiles for the h0 streams load first (first gather needs
            # them at ~50us), then the zero-fills (first scatter per table),
            # then the h1 idx tiles -- all on the Activation HWDGE queue.
            gis = {}
            sis = {}

            def load_idx(h):
                for d in range(C):
                    S = NPAD[h][d] // 16
                    gi = idxp.tile([P, S], I16, tag=f"gi{h}_{d}")
                    nc.scalar.dma_start(out=gi[:], in_=gaps[h][d][:, :])
                    si = idxp.tile([P, S], I16, tag=f"si{h}_{d}")
                    nc.scalar.dma_start(out=si[:], in_=saps[h][d][:, :])
                    gis[(h, d)] = gi
                    sis[(h, d)] = si

            zero = const.tile([P, Np], F16)
            nc.vector.memset(zero[:], 0.0)
            load_idx(0)
            load_idx(1)

            # dummy 16-idx gather: forces the GPSIMD library overlay load to
            # the head of the DMA queues (otherwise the first real gather
            # stalls ~85us behind the whole down-projection DMA stream)
            dummy_i = const.tile([P, 1], I16)
            nc.vector.memset(dummy_i[:], 0)
            dummy_g = const.tile([P, OUT], F16)
            nc.gpsimd.dma_gather(
                out_ap=dummy_g[:].rearrange("p (e f) -> p e f", f=OUT),
                in_ap=diag[:, 0:P],
                idxs_ap=dummy_i[:],
                num_idxs=16,
                num_idxs_reg=16,
                elem_size=OUT,
                elem_step=W * P,
            )

            # persistent SBUF constants
            wd_sb = const.tile([P, cfg.KIN * OUT], F16)
            nc.sync.dma_start(out=wd_sb[:], in_=wd[:, :])
            wgu_sb = const.tile([OUT, IN], F16)
            nc.sync.dma_start(out=wgu_sb[:], in_=wgu[:, :])
            bu2_sb = const.tile([P, IN], F16)
            nc.sync.dma_start(out=bu2_sb[:], in_=bu2[:, :])
            ones_sb = const.tile([P, P], F16)
            nc.vector.memset(ones_sb[:], 1.0)

            # ---- down-projection (h_lo windows first, then h_hi) ----
            assert WLO % HST == 0 and W % HST == 0
            for g0 in range(0, W, HST):
                gw = min(HST, W - g0)
                h = int(g0 >= WLO)
                w_base = WLO if h else 0
                ht = hsb.tile([P, gw * OUT], F16)
                for k in range(gw):
                    w = g0 + k
                    xt = xap.tile([P, cfg.KIN * P], F16)
                    nc.sync.dma_start(
                        out=xt[:],
                        in_=xa[:, w * cfg.KIN * P:(w + 1) * cfg.KIN * P],
                    )
                    ps = dpsum.tile([P, OUT], F32, space="PSUM")
                    for cc in range(cfg.KC):
                        nc.tensor.matmul(
                            ps[:],
                            lhsT=xt[:, cc * P:(cc + 1) * P],
                            rhs=wd_sb[:, cc * OUT:(cc + 1) * OUT],
                            start=(cc == 0),
                            stop=False,
                        )
                    nc.tensor.matmul(
                        ps[:],
                        lhsT=xt[0:1, cfg.KC * P:cfg.KC * P + P],
                        rhs=wd_sb[0:1, cfg.KC * OUT:cfg.KC * OUT + OUT],
                        start=False,
                        stop=True,
                    )
                    nc.vector.tensor_copy(out=ht[:, k * OUT:(k + 1) * OUT], in_=ps[:])
                nc.sync.dma_start(
                    out=h_half[h][(g0 - w_base) * P:(g0 - w_base + gw) * P, :]
                        .rearrange("(w p) f -> p w f", p=P),
                    in_=ht[:].rearrange("p (w f) -> p w f", f=OUT),
                )

            # ---- gather + scatter-add streams (h_lo streams run while the
            # down-projection of h_hi is still in flight) ----
            for h in range(2):
                for d in range(C):
                    gi = gis[(h, d)]
                    si = sis[(h, d)]
                    if h == 0:
                        # zero-fill table d just-in-time: emitting the zero
                        # copies any earlier poisons the shared rotating DMA
                        # sems and stalls the first gathers behind them
                        nc.scalar.dma_start(
                            out=aggs[d * Np:(d + 1) * Np, :], in_=zero[:]
                        )

                    # round boundaries (128-aligned)
                    r_offs = []
                    off = 0
                    for size in RS_SIZES[h][d]:
                        r_offs.append((off, off + size))
                        off += size

                    for c_lo in range(0, NPAD[h][d], CHUNK):
                        c_hi = min(c_lo + CHUNK, NPAD[h][d])
                        n = c_hi - c_lo
                        G = gsb.tile([P, (n // P) * OUT], F16)
                        nc.gpsimd.dma_gather(
                            out_ap=G[:].rearrange("p (e f) -> p e f", f=OUT),
                            in_ap=h_half[h][:],
                            idxs_ap=gi[:, c_lo // 16:c_hi // 16],
                            num_idxs=n,
                            num_idxs_reg=n,
                            elem_size=OUT,
                        )
                        # scatter (round ∩ chunk) slices: idx-unique per inst
                        for (r_lo, r_hi) in r_offs:
                            s_lo = max(r_lo, c_lo)
                            s_hi = min(r_hi, c_hi)
                            for lo in range(s_lo, s_hi, SCAP):
                                hi = min(lo + SCAP, s_hi)
                                m = hi - lo
                                nc.gpsimd.dma_scatter_add(
                                    out_ap=aggs[d * Np:(d + 1) * Np, :],
                                    in_ap=G[:, ((lo - c_lo) // P) * OUT:((hi - c_lo) // P) * OUT]
                                        .rearrange("p (e f) -> p e f", f=OUT),
                                    idxs_ap=si[:, lo // 16:hi // 16],
                                    num_idxs=m,
                                    num_idxs_reg=m,
                                    elem_size=OUT,
                                )

            # diag is only needed by the tail; late emission keeps its 9.7us
            # load from gating phase-2 waits on the shared rotating sems
            diag_sb = const.tile([P, W * P], F16)
            nc.scalar.dma_start(out=diag_sb[:], in_=diag[:, :])

            # ---- reduce-scatter the partials ----
            nc.gpsimd.collective_compute(
                "ReduceScatter",
                mybir.AluOpType.add,
                replica_groups=[list(range(C))],
                ins=[aggs[:].opt()],
                outs=[rs_out[:].opt()],
            )

            # ---- tail: norm_dst (via diag transpose), up-projection, bias ----
            for g0 in range(0, W, HST):
                gw = min(HST, W - g0)
                aw = asb.tile([P, gw * OUT], F16)
                nc.sync.dma_start(
                    out=aw[:].rearrange("p (w f) -> p w f", f=OUT),
                    in_=rs_out[g0 * P:(g0 + gw) * P, :]
                        .rearrange("(w p) f -> p w f", p=P),
                )
                for k in range(gw):
                    w = g0 + k
                    tps = tpsum.tile([P, OUT], F32, space="PSUM")
                    # aggT[f, j] = agg[j, f] * norm_dst[j]
                    nc.tensor.matmul(
                        tps[:],
                        lhsT=aw[:, k * OUT:(k + 1) * OUT],
                        rhs=diag_sb[:, w * P:(w + 1) * P],
                        start=True,
                        stop=True,
                    )
                    aggT = atb.tile([P, OUT], F16)
                    nc.vector.tensor_copy(out=aggT[:], in_=tps[:])
                    ops = opsum.tile([P, IN], F32, space="PSUM")
                    for lo in range(0, IN, 512):
                        hi = min(lo + 512, IN)
                        nc.tensor.matmul(
                            ops[:, lo:hi],
                            lhsT=ones_sb[0:1, 0:P],
                            rhs=bu2_sb[0:1, lo:hi],
                            start=True,
                            stop=False,
                        )
                        nc.tensor.matmul(
                            ops[:, lo:hi],
                            lhsT=aggT[:],
                            rhs=wgu_sb[:, lo:hi],
                            start=False,
                            stop=True,
                        )
                    ot = osb.tile([P, IN], F16)
                    nc.scalar.copy(out=ot[:, :IN // 2], in_=ops[:, :IN // 2])
                    nc.vector.tensor_copy(out=ot[:, IN // 2:], in_=ops[:, IN // 2:])
                    nc.sync.dma_start(out=out[w * P:(w + 1) * P, :], in_=ot[:])

    nc.compile()
    return nc


def assemble_output(cfg, outs, node_core, node_slot):
    allo = np.stack([np.asarray(o) for o in outs])
    return allo[node_core, node_slot, :].astype(np.float32)


_GRAPH_CACHE = {}


def kernel(features, Wd, bd, Wg, bg, Wu, bu, src, dst):
    features = np.asarray(features)
    N, IN = features.shape
    OUT = np.asarray(Wd).shape[1]
    E = np.asarray(src).shape[0]
    cfg = Cfg(N, E, IN, OUT)

    in_maps, node_core, node_slot, shared = host_prep(
        cfg, features, Wd, bd, Wg, bg, Wu, bu, src, dst
    )
    key = (N, E, IN, OUT, shared[0], shared[1])
    nc = _GRAPH_CACHE.get(key)
    if nc is None:
        nc = build_graph(cfg, shared)
        _GRAPH_CACHE[key] = nc

    res = run_bass_kernel_spmd(nc, in_maps, core_ids=list(range(cfg.C)))
    outs = [res.results[i]["out"] for i in range(cfg.C)]
    return assemble_output(cfg, outs, node_core, node_slot)
